# revision 16
# baseline (speedup 1.0000x reference)
"""MoE top-2 routing kernel for Trainium2 (8 NeuronCores, expert-parallel).

Reference computes a dense MoE (all E=8 experts over all T=4096 tokens) then
combines with a top-2 sparse mixture matrix. Only the top-2 experts per token
contribute to the output, so we:

  1. (host) compute gate logits / gumbel softmax / top-2 routing + the
     load-balance loss (~0.05% of total FLOPs),
  2. (host) gather each expert's tokens into a padded, transposed activation
     block xT[e] = x[idx_e].T  with a common capacity N_cap,
  3. (device, expert-parallel) core e runs expert e's FFN:
         yT = w2.T @ gelu(w1.T @ xT + b1) + b2
     with weights used in native layout as the stationary matmul operand,
  4. (host) scatter-add  out[idx_e] += gate_val_e * yT.T  over the 8 experts.

Layouts on device (everything feature-major so no transposes are needed):
  xT   [C=1024, N_cap]   tokens of this expert, transposed
  w1   [C=1024, H=4096]  native
  w2   [H=4096, C=1024]  native
  hT   [H, chunk]        intermediate, SBUF-resident per token chunk
  yT   [C=1024, N_cap]   output, transposed

Matmuls run as  out[M,N] = lhsT[K,M].T @ rhs[K,N]  with M=128 feature tile,
N = token chunk (moving dim), K=128 contraction tile:
  MM1: lhsT = w1[k*128:, j*128:]  rhs = xT[k]   -> hT[j] (accum over k=0..7)
  MM2: lhsT = w2[k*128:, c*128:]  rhs = hT[k]   -> yT[c] (accum over k=0..31)
"""

import os
import numpy as np

B, N, C, H, E, K = 2, 2048, 1024, 4096, 8, 2
TAU = 1.0
NCORES = 8

# Token chunk size (matmul moving dim). fp32 PSUM bank holds 512 fp32;
# fp32r needs >=256 moving for full rate. 384 gives finer capacity
# granularity (less padding) while staying full-rate.
TC = 384

# matmul input dtype: "fp32r" (relaxed fp32, full rate) | "fp32" | "bf16"
MM_DTYPE = os.environ.get("MOE_MM_DTYPE", "fp32r")

LAST_RUN_INFO = {}


def _routing(x_flat, gumbel_noise, gate_w, gate_b):
    """Reproduce the reference gating numerics (float64 for value accuracy;
    routing decisions are identical to the f32 reference except for
    ~1e-7-degenerate ties, where the output difference is O(1e-7) anyway)."""
    logits = x_flat.astype(np.float64) @ gate_w.astype(np.float64)
    logits += gate_b.astype(np.float64)
    z = (logits + gumbel_noise.astype(np.float64)) / TAU
    z -= z.max(axis=-1, keepdims=True)
    ez = np.exp(z)
    gates = ez / ez.sum(axis=-1, keepdims=True)          # [T, E]
    top2 = np.argsort(-gates, axis=-1, kind="stable")[:, :K]  # [T, K]
    mean_gates = gates.mean(axis=0)
    loss = np.float32(np.sum(mean_gates * np.log(mean_gates + 1e-8)))
    return gates, top2, loss


def _build_bass(n_cap, mm_dtype, act="gelu", reps=1):
    import concourse.bass as bass
    import concourse.mybir as mybir
    import concourse.tile as tile

    fp32 = mybir.dt.float32
    if mm_dtype == "fp32r":
        mmdt = mybir.dt.float32r
        io_np = np.float32
    elif mm_dtype == "fp32":
        mmdt = mybir.dt.float32
        io_np = np.float32
    elif mm_dtype == "bf16":
        mmdt = mybir.dt.bfloat16
        import ml_dtypes
        io_np = ml_dtypes.bfloat16
    else:
        raise ValueError(mm_dtype)

    n_chunks = n_cap // TC
    assert n_cap % TC == 0

    import concourse.bacc as bacc
    nc = bacc.Bacc(None, target_bir_lowering=False)
    # All matmul inputs are declared directly in the matmul dtype (float32r
    # is byte-identical to float32 host-side).
    xT_d = nc.declare_dram_parameter("xT", [C, n_cap], mmdt, isOutput=False)
    w1_d = nc.declare_dram_parameter("w1", [C, H], mmdt, isOutput=False)
    w2_d = nc.declare_dram_parameter("w2", [H, C], mmdt, isOutput=False)
    b1_d = nc.declare_dram_parameter("b1", [H], fp32, isOutput=False)
    b2_d = nc.declare_dram_parameter("b2", [C], fp32, isOutput=False)
    yT_d = nc.declare_dram_parameter("yT", [C, n_cap], fp32, isOutput=True)

    P = 128
    CK = C // P            # 8   C contraction tiles
    HK = H // P            # 32  H tiles
    HG = 4                 # H tiles per MM1 psum group (psum1 bufs)
    CG = 4                 # C tiles per MM2 psum group (psum2 bufs)

    xT_r = xT_d.rearrange("(o p) t -> p o t", p=P)    # [128, 8, n_cap]
    yT_r = yT_d.rearrange("(o p) t -> p o t", p=P)
    w1_r = w1_d.rearrange("(o p) h -> p o h", p=P)    # [128, 8, 4096]
    w2_r = w2_d.rearrange("(o p) c -> p o c", p=P)    # [128, 32, 1024]
    b1_r = b1_d.rearrange("(o p) -> p o", p=P)        # [128, 32]
    b2_r = b2_d.rearrange("(o p) -> p o", p=P)        # [128, 8]

    gelu = {
        "gelu": mybir.ActivationFunctionType.Gelu,
        "relu": mybir.ActivationFunctionType.Relu,  # for CoreSim validation only
    }[act]
    ident = mybir.ActivationFunctionType.Identity

    HGT = 4    # H tiles per MM1 weight slab group (512 H cols)
    CGT = 2    # C tiles per MM2 weight slab group (256 C cols)

    from concourse.tile_rust import add_dep_helper

    with tile.TileContext(nc) as tc:
        with (
            tc.tile_pool(name="const", bufs=1) as const_pool,
            tc.tile_pool(name="xp", bufs=2) as x_pool,
            tc.tile_pool(name="w1p", bufs=2) as w1_pool,
            tc.tile_pool(name="w2p", bufs=2) as w2_pool,
            tc.tile_pool(name="hp", bufs=1) as h_pool,
            tc.tile_pool(name="yp", bufs=4) as y_pool,
            tc.tile_pool(name="ps1p", bufs=4, space="PSUM") as ps1,
            tc.tile_pool(name="ps2p", bufs=3, space="PSUM") as ps2,
            tc.tile_pool(name="psdp", bufs=1, space="PSUM") as psd,
        ):
            b1_sb = const_pool.tile([P, HK], fp32, name="b1_sb")
            nc.sync.dma_start(out=b1_sb, in_=b1_r)
            b2_sb = const_pool.tile([P, CK], fp32, name="b2_sb")
            nc.sync.dma_start(out=b2_sb, in_=b2_r)

            # Wait-absorber machinery: walrus allows at most ONE semaphore
            # wait on a (self-loading) Matmult. A tiny PE matmul reading each
            # freshly-DMA'd slab absorbs the DMA-queue wait so the real
            # matmuls carry at most one remaining wait (psum release / ACT).
            dmy_sb = const_pool.tile([P, P], mmdt, name="dmy_sb")
            nc.any.memzero(dmy_sb)

            def absorb(dep_ap):
                ps = psd.tile([P, 8], fp32, name="dmy_ps")
                return nc.tensor.matmul(
                    ps, lhsT=dep_ap, rhs=dmy_sb[:, 0:8], start=True, stop=True
                )

            # bootstrap: make PE observe dmy_sb's writer once
            absorb(dmy_sb)

            for ci in range(n_chunks * reps):
                ci = ci % n_chunks
                tok = slice(ci * TC, (ci + 1) * TC)
                xc = x_pool.tile([P, CK, TC], mmdt, name="xc")
                nc.sync.dma_start(out=xc, in_=xT_r[:, :, tok])
                dep = absorb(xc[:, 0, 0:P])
                hT = h_pool.tile([P, HK, TC], mmdt, name="hT")

                # ---- MM1: hT[jj] = gelu(sum_k w1[k,jj].T @ x[k] + b1[jj])
                for g in range(HK // HGT):         # 8 slab groups
                    w1g = w1_pool.tile([P, CK, HGT * P], mmdt, name="w1g")
                    nc.sync.dma_start(
                        out=w1g, in_=w1_r[:, :, g * HGT * P:(g + 1) * HGT * P]
                    )
                    dep = absorb(w1g[:, 0, 0:P])
                    for j in range(HGT):
                        jj = g * HGT + j
                        ps = ps1.tile([P, TC], fp32, name="ps1")
                        for k in range(CK):
                            mm = nc.tensor.matmul(
                                ps,
                                lhsT=w1g[:, k, j * P:(j + 1) * P],
                                rhs=xc[:, k],
                                start=(k == 0),
                                stop=(k == CK - 1),
                            )
                            if j == 0 and k == 0:
                                add_dep_helper(
                                    dep.ins, mm.ins,
                                    sync=False, reason="absorb-order",
                                )
                        nc.scalar.activation(
                            out=hT[:, jj], in_=ps, func=gelu,
                            bias=b1_sb[:, jj:jj + 1],
                        )

                # ---- MM2: yT[cc] = sum_k w2[k,cc].T @ hT[k] + b2[cc]
                for g in range(CK // CGT):         # 4 slab groups
                    w2g = w2_pool.tile([P, HK, CGT * P], mmdt, name="w2g")
                    nc.sync.dma_start(
                        out=w2g, in_=w2_r[:, :, g * CGT * P:(g + 1) * CGT * P]
                    )
                    dep = absorb(w2g[:, 0, 0:P])
                    for c in range(CGT):
                        cc = g * CGT + c
                        ps = ps2.tile([P, TC], fp32, name="ps2")
                        for k in range(HK):
                            mm = nc.tensor.matmul(
                                ps,
                                lhsT=w2g[:, k, c * P:(c + 1) * P],
                                rhs=hT[:, k],
                                start=(k == 0),
                                stop=(k == HK - 1),
                            )
                            if c == 0 and k == 0:
                                add_dep_helper(
                                    dep.ins, mm.ins,
                                    sync=False, reason="absorb-order",
                                )
                        yt = y_pool.tile([P, TC], fp32, name="yt")
                        nc.scalar.activation(
                            out=yt, in_=ps, func=ident,
                            bias=b2_sb[:, cc:cc + 1],
                        )
                        nc.sync.dma_start(out=yT_r[:, cc, tok], in_=yt)

    return nc, io_np


def kernel(x, gumbel_noise, gate_w, gate_b, w1, b1, w2, b2):
    from concourse.bass_utils import run_bass_kernel_spmd

    x = np.asarray(x)
    x_flat = np.ascontiguousarray(x.reshape(-1, C))
    T = x_flat.shape[0]

    gates, top2, loss = _routing(
        x_flat, np.asarray(gumbel_noise), np.asarray(gate_w), np.asarray(gate_b)
    )

    # token index lists per expert (order within expert irrelevant)
    idxs = [np.where((top2 == e).any(axis=1))[0] for e in range(E)]
    counts = np.array([len(i) for i in idxs])
    n_cap = int(max(512, -(-counts.max() // TC) * TC))

    reps = int(os.environ.get("MOE_REPS", "1"))
    nc, io_np = _build_bass(n_cap, MM_DTYPE, reps=reps)
    nc.finalize()

    w1 = np.asarray(w1)
    w2 = np.asarray(w2)
    b1 = np.asarray(b1)
    b2 = np.asarray(b2)

    in_maps = []
    for e in range(E):
        xe = np.zeros((C, n_cap), dtype=np.float32)
        xe[:, : counts[e]] = x_flat[idxs[e]].T
        in_maps.append({
            "xT": xe.astype(io_np),
            "w1": np.ascontiguousarray(w1[e]).astype(io_np),
            "w2": np.ascontiguousarray(w2[e]).astype(io_np),
            "b1": np.ascontiguousarray(b1[e]).astype(np.float32),
            "b2": np.ascontiguousarray(b2[e]).astype(np.float32),
        })

    trace = bool(int(os.environ.get("MOE_TRACE", "0")))
    kwargs = {}
    if trace:
        tmpdir = os.environ.get("MOE_TRACE_DIR") or None
        kwargs = dict(trace=True, tmpdir=tmpdir)
    import time as _time
    runs = int(os.environ.get("MOE_RUNS", "1"))
    run_walls = []
    res = None
    for _ in range(runs):
        t0 = _time.time()
        try:
            res = run_bass_kernel_spmd(nc, in_maps, list(range(NCORES)), **kwargs)
        except Exception:
            if not kwargs:
                raise
            kwargs = {}
            res = run_bass_kernel_spmd(nc, in_maps, list(range(NCORES)))
        run_walls.append(_time.time() - t0)

    LAST_RUN_INFO.clear()
    LAST_RUN_INFO.update(
        exec_time_ns=res.exec_time_ns,
        profile_json=res.profile_json,
        n_cap=n_cap,
        counts=counts,
        reps=reps,
        run_walls=run_walls,
    )

    # host combine: out[t] = sum over the token's top-2 experts of g_e * y_e[t]
    out_flat = np.zeros((T, C), dtype=np.float32)
    for e in range(E):
        yT = np.asarray(res.results[e]["yT"])            # [C, n_cap] f32
        ge = gates[idxs[e], e].astype(np.float32)        # [n_e]
        out_flat[idxs[e]] += ge[:, None] * yT[:, : counts[e]].T
    out = out_flat.reshape(x.shape[0], x.shape[1], C)
    return out, loss
